# revision 14
# baseline (speedup 1.0000x reference)
"""GCN (3-layer, PyG-style symmetric norm) on 8 Trainium2 NeuronCores.

Strategy (hardcoded for N=50000, E=800000, C=128, 8 cores):
  - Nodes sharded by contiguous ranges of 6250 across 8 cores; edges
    partitioned by dst so segment-sum is local to the dst owner.
  - Aggregation per dst block b is a chain of TensorE scatter matmuls:
    aggT_b[ch, col] += G_chunk[slot, ch]^T E_chunk[slot, col], where
    E (host-precomputed, streamed from HBM) carries the edge norms.
  - Layer 1 needs no gather/AllGather: sources are rows of the input x,
    so the host pre-duplicates x rows into chunk-slot order (xg) and
    the scatter runs in input space; W1 is applied per block afterwards
    (linearity), self-loops via a diag matmul on t1 = dinv*(x@W1).
  - Layer 2 computes the table slab t2 = dinv*(h1 @ W2), AllGathers it,
    and gathers source rows with dma_gather (int16 idx; lo/hi halves).
  - Layer 3 collapses algebraically: the final output is a mean over
    all nodes, so out = (sum_n q_n h2[n]) @ W3 / N + b3 with
    host-computed q_n = dinv_n^2 (dinv_n + sum_{e:src=n} dinv_dst_e).
    Only a weighted column-reduce of h2 plus one tiny W3 matmul.
"""

import sys

for _p in ("/opt/trn_rl_repo", "/root/.axon_site/_ro/trn_rl_repo"):
    if _p not in sys.path:
        sys.path.insert(0, _p)

import numpy as np


class GCNConfig:
    """Node ownership: core r owns lo-range [r*LO_PER, (r+1)*LO_PER) and
    hi-range [SPLIT + r*HI_PER, SPLIT + (r+1)*HI_PER).  SPLIT = M*LO_PER
    keeps both gather tables int16-addressable and offset-free."""

    def __init__(self, n_nodes=50000, n_edges=800000, n_cores=8,
                 lo_per_core=4096, blocks_per_group=4, blocks_per_group1=2):
        assert n_nodes % n_cores == 0
        self.N = n_nodes
        self.E = n_edges
        self.C = 128
        self.M = n_cores
        self.LN = n_nodes // n_cores          # local nodes per core
        self.LO_PER = min(lo_per_core, self.LN)
        self.SPLIT = self.LO_PER * n_cores
        self.HI_PER = self.LN - self.LO_PER
        assert self.LO_PER <= 32768 and self.HI_PER <= 32767
        self.NBLK_LO = -(-self.LO_PER // 128)
        self.NBLK_HI = -(-self.HI_PER // 128) if self.HI_PER else 0
        self.NBLK = self.NBLK_LO + self.NBLK_HI
        self.NPAD = self.NBLK * 128           # padded local node count
        self.GB = blocks_per_group            # blocks per gather group (L2)
        self.GB1 = blocks_per_group1          # blocks per stream group (L1)
        assert self.HI_PER == 0 or self.LO_PER % 128 == 0

    def storage_range(self, r):
        """Storage positions of core r's local ordering [0, LN)."""
        g = np.empty(self.LN, dtype=np.int64)
        g[:self.LO_PER] = r * self.LO_PER + np.arange(self.LO_PER)
        if self.HI_PER:
            g[self.LO_PER:] = (self.SPLIT + r * self.HI_PER
                               + np.arange(self.HI_PER))
        return g


def _balance_positions(cfg, w_node):
    """Assign nodes to storage positions, balancing per-(core, block) edge
    loads within each half. Returns pos[node] -> storage position."""
    import heapq
    N, M, SPLIT = cfg.N, cfg.M, cfg.SPLIT
    pos = np.empty(N, dtype=np.int64)
    for half in (0, 1):
        if half == 0:
            ids = np.arange(0, SPLIT)
            nblk, per = cfg.NBLK_LO, cfg.LO_PER
            base = 0
        else:
            if cfg.HI_PER == 0:
                break
            ids = np.arange(SPLIT, N)
            nblk, per = cfg.NBLK_HI, cfg.HI_PER
            base = SPLIT
        bins = []
        cap = {}
        fill = {}
        for r in range(M):
            for b in range(nblk):
                w = min(128, per - b * 128)
                bins.append((0.0, (r, b)))
                cap[(r, b)] = w
                fill[(r, b)] = []
        heapq.heapify(bins)
        order = ids[np.argsort(-w_node[ids], kind="stable")]
        for n in order:
            while True:
                load, key = heapq.heappop(bins)
                if len(fill[key]) < cap[key]:
                    break
            fill[key].append(n)
            if len(fill[key]) < cap[key]:
                heapq.heappush(bins, (load + float(w_node[n]), key))
        for (r, b), members in fill.items():
            start = base + r * per + b * 128
            for i, n in enumerate(members):
                pos[n] = start + i
    return pos


def host_prep(cfg, x, edge_index, dinv=None):
    """Build per-core input arrays + the shared chunk schedules."""
    N, M, LN, NBLK, SPLIT = cfg.N, cfg.M, cfg.LN, cfg.NBLK, cfg.SPLIT

    src0 = np.asarray(edge_index[0], dtype=np.int64)
    dst0 = np.asarray(edge_index[1], dtype=np.int64)

    if dinv is None:
        deg = (np.bincount(dst0, minlength=N) + 1).astype(np.float32)
        dinv = (1.0 / np.sqrt(deg)).astype(np.float32)

    # balance (core, block) bin loads; nodes keep their half
    w_node = np.bincount(dst0, minlength=N).astype(np.float64)
    pos = _balance_positions(cfg, w_node)
    inv = np.empty(N, dtype=np.int64)
    inv[pos] = np.arange(N)
    dinv_stor = dinv[inv]                     # dinv by storage position

    src_all = pos[src0]
    dst_all = pos[dst0]

    # dst position -> (owner core, local index) under the lo/hi ownership
    is_hi_dst = dst_all >= SPLIT
    q = dst_all - SPLIT
    core = np.where(is_hi_dst, q // max(cfg.HI_PER, 1), dst_all // cfg.LO_PER)
    li = np.where(is_hi_dst, cfg.LO_PER + q % max(cfg.HI_PER, 1),
                  dst_all % cfg.LO_PER)
    blk = li // 128
    dloc = li % 128
    half = (src_all >= SPLIT).astype(np.int64)

    # ---- layer-2 schedule: (core, block, half) runs
    key = (core * NBLK + blk) * 2 + half
    order = np.argsort(key, kind="stable")
    s_src = src_all[order]
    s_dloc = dloc[order]
    counts = np.bincount(key, minlength=M * NBLK * 2).reshape(M, NBLK, 2)
    starts = np.zeros(M * NBLK * 2 + 1, dtype=np.int64)
    np.cumsum(counts.reshape(-1), out=starts[1:])

    CL = ((counts[:, :, 0] + 127) // 128).max(axis=0)
    CH = ((counts[:, :, 1] + 127) // 128).max(axis=0)

    groups = [list(range(g, min(g + cfg.GB, NBLK)))
              for g in range(0, NBLK, cfg.GB)]
    schedule = []  # (block, half)
    for grp in groups:
        for b in grp:
            schedule += [(b, 0)] * int(CL[b])
        for b in grp:
            schedule += [(b, 1)] * int(CH[b])
    NCHUNK = len(schedule)

    # ---- layer-1 schedule: (core, block) runs (halves merged; no gather)
    key1 = core * NBLK + blk
    order1 = np.argsort(key1, kind="stable")
    s1_src = src_all[order1]                  # global storage position
    s1_dloc = dloc[order1]
    counts1 = counts.sum(axis=2)              # [M, NBLK]
    starts1 = np.zeros(M * NBLK + 1, dtype=np.int64)
    np.cumsum(counts1.reshape(-1), out=starts1[1:])
    C1 = ((counts1 + 127) // 128).max(axis=0)

    groups1 = [list(range(g, min(g + cfg.GB1, NBLK)))
               for g in range(0, NBLK, cfg.GB1)]
    schedule1 = []  # block
    for grp in groups1:
        for b in grp:
            schedule1 += [b] * int(C1[b])
    NCHUNK1 = len(schedule1)

    # x by storage position (for layer-1 host pre-gather)
    x_stor = np.zeros((SPLIT + max(N - SPLIT, 0), 128), dtype=np.float32)
    x_stor[pos] = np.asarray(x, dtype=np.float32)

    # layer-3 collapse coefficients: q_n = dinv_n (dinv_n + sum dinv_dst)
    csum = np.bincount(src0, weights=dinv[dst0].astype(np.float64),
                       minlength=N)
    q_node = (dinv.astype(np.float64)
              * (dinv.astype(np.float64) + csum)).astype(np.float32)
    q_stor = q_node[inv]

    cols = np.arange(128, dtype=np.float32)
    per_core = []
    for r in range(M):
        # ---- layer-2 idx + E arrays
        idx_arr = np.zeros((NCHUNK, 128), dtype=np.int64)
        dl_arr = np.full((NCHUNK, 128), -1.0, dtype=np.float32)
        pos_in = {}
        for ci, (b, h) in enumerate(schedule):
            k = pos_in.get((b, h), 0)
            pos_in[(b, h)] = k + 1
            kk = (r * NBLK + b) * 2 + h
            lo, hi = starts[kk], starts[kk + 1]
            a = lo + k * 128
            nreal = max(0, min(128, hi - a))
            if nreal > 0:
                seg = slice(a, a + nreal)
                sv = s_src[seg]
                idx_arr[ci, :nreal] = sv - (SPLIT if h else 0)
                dl_arr[ci, :nreal] = s_dloc[seg]
        flat = idx_arr.reshape(-1)
        w16 = flat.reshape(-1, 16).T.astype(np.int16)  # [16, NCHUNK*8]
        idxw = np.tile(w16, (8, 1))                    # [128, NCHUNK*8]

        g = inv[cfg.storage_range(r)]
        dv = np.zeros(cfg.NPAD, dtype=np.float32)
        dv[:LN] = dinv[g]
        dinvc = np.ascontiguousarray(dv.reshape(NBLK, 128).T)  # [128, NBLK]

        # layer-2 E: E[slot, j] = dinv_dst  (src dinv lives in the table)
        blks = np.array([b for (b, h) in schedule])
        dvblk = dinvc.T[blks]                        # [NCHUNK, 128]
        E = (dl_arr[:, :, None] == cols[None, None, :]).astype(np.float16)
        E *= dvblk[:, None, :].astype(np.float16)
        eall = np.ascontiguousarray(
            E.transpose(1, 0, 2).reshape(128, NCHUNK * 128))

        # ---- layer-1 xg + E1 arrays (chunk-slot order, x rows duplicated)
        src1_arr = np.zeros((NCHUNK1, 128), dtype=np.int64)
        dl1_arr = np.full((NCHUNK1, 128), -1.0, dtype=np.float32)
        sc1_arr = np.zeros((NCHUNK1, 128), dtype=np.float32)  # dinv_src
        pos_in1 = {}
        for ci, b in enumerate(schedule1):
            k = pos_in1.get(b, 0)
            pos_in1[b] = k + 1
            kk = r * NBLK + b
            lo, hi = starts1[kk], starts1[kk + 1]
            a = lo + k * 128
            nreal = max(0, min(128, hi - a))
            if nreal > 0:
                seg = slice(a, a + nreal)
                src1_arr[ci, :nreal] = s1_src[seg]
                dl1_arr[ci, :nreal] = s1_dloc[seg]
                sc1_arr[ci, :nreal] = dinv_stor[s1_src[seg]]
        # xg: [slot, chunk, ch] -> [128, NCHUNK1*128] (pure row duplication)
        xg = x_stor[src1_arr.reshape(-1)].astype(np.float16)
        xg = np.ascontiguousarray(
            xg.reshape(NCHUNK1, 128, 128).transpose(1, 0, 2)
            .reshape(128, NCHUNK1 * 128))
        # E1: full norm dinv_src * dinv_dst
        blks1 = np.array(schedule1)
        dvblk1 = dinvc.T[blks1]                      # [NCHUNK1, 128]
        E1 = (dl1_arr[:, :, None] == cols[None, None, :]).astype(np.float32)
        E1 *= sc1_arr[:, :, None] * dvblk1[:, None, :]
        e1all = np.ascontiguousarray(
            E1.astype(np.float16).transpose(1, 0, 2)
            .reshape(128, NCHUNK1 * 128))

        # self-loop diag(dinv) per block
        sd = np.zeros((128, NBLK * 128), dtype=np.float16)
        for b in range(NBLK):
            np.fill_diagonal(sd[:, b * 128:(b + 1) * 128],
                             dinvc[:, b].astype(np.float16))

        # layer-3 weighted-reduce coefficients, broadcast across partitions
        qv = np.zeros(cfg.NPAD, dtype=np.float32)
        qv[:LN] = q_stor[cfg.storage_range(r)]
        qdall = np.ascontiguousarray(
            np.broadcast_to(qv[None, :], (128, cfg.NPAD)))

        xs = x_stor[cfg.storage_range(r)]
        xT = np.zeros((128, cfg.NPAD), dtype=np.float32)
        xT[:, :LN] = xs.T

        per_core.append(dict(xT=xT, idxw=idxw, dinvc=dinvc, eall=eall,
                             xg=xg, e1all=e1all, sd=sd, qdall=qdall))

    meta = dict(CL=CL.astype(int).tolist(), CH=CH.astype(int).tolist(),
                C1=C1.astype(int).tolist(), groups=groups, groups1=groups1,
                schedule=schedule, schedule1=schedule1,
                NCHUNK=NCHUNK, NCHUNK1=NCHUNK1)
    return meta, per_core, dinv


SINGLE_PACKET = True


def build_program(cfg, meta):
    import concourse.bass as bass
    import concourse.bacc as bacc
    import concourse.tile as tile
    from concourse import mybir
    from contextlib import ExitStack

    f32 = mybir.dt.float32
    f16 = mybir.dt.float16
    i16 = mybir.dt.int16
    N, C, M, LN = cfg.N, cfg.C, cfg.M, cfg.LN
    NBLK, NPAD, SPLIT = cfg.NBLK, cfg.NPAD, cfg.SPLIT

    def blk_width(b):
        if b < cfg.NBLK_LO:
            return min(128, cfg.LO_PER - b * 128)
        return min(128, cfg.HI_PER - (b - cfg.NBLK_LO) * 128)

    def blk_slab_row(b):
        if b < cfg.NBLK_LO:
            return 0, b * 128
        return 1, (b - cfg.NBLK_LO) * 128

    CL, CH, C1 = meta["CL"], meta["CH"], meta["C1"]
    groups, groups1 = meta["groups"], meta["groups1"]
    schedule, schedule1 = meta["schedule"], meta["schedule1"]
    NCHUNK, NCHUNK1 = meta["NCHUNK"], meta["NCHUNK1"]

    total_chunks = [CL[b] + CH[b] for b in range(NBLK)]
    last_lo_grp1 = max(gi for gi, grp in enumerate(groups1)
                       if any(b < cfg.NBLK_LO for b in grp))
    last_grp1 = len(groups1) - 1

    nc = bacc.Bacc(None, target_bir_lowering=False, debug=False)
    xT_e = nc.declare_dram_parameter("xT", [128, NPAD], f32, isOutput=False)
    idx_e = nc.declare_dram_parameter("idxw", [128, NCHUNK * 8], i16,
                                      isOutput=False)
    dinvc_e = nc.declare_dram_parameter("dinvc", [128, NBLK], f32,
                                        isOutput=False)
    eall_e = nc.declare_dram_parameter("eall", [128, NCHUNK * 128], f16,
                                       isOutput=False)
    xg_e = nc.declare_dram_parameter("xg", [128, NCHUNK1 * 128], f16,
                                     isOutput=False)
    e1all_e = nc.declare_dram_parameter("e1all", [128, NCHUNK1 * 128], f16,
                                        isOutput=False)
    sd_e = nc.declare_dram_parameter("sd", [128, NBLK * 128], f16,
                                     isOutput=False)
    qdall_e = nc.declare_dram_parameter("qdall", [128, NPAD], f32,
                                        isOutput=False)
    W_e = [nc.declare_dram_parameter(f"W{i}", [C, C], f32, isOutput=False)
           for i in (1, 2, 3)]
    b_e = [nc.declare_dram_parameter(f"b{i}", [C, 1], f32, isOutput=False)
           for i in (1, 2)]
    out_e = nc.declare_dram_parameter("out_vec", [1, C], f32, isOutput=True)

    with TileCtx(nc, tile) as tc, ExitStack() as ctx:
        const = ctx.enter_context(tc.tile_pool(name="const", bufs=1))
        dram = ctx.enter_context(tc.tile_pool(name="dram", bufs=1,
                                              space="DRAM"))
        gpool = ctx.enter_context(tc.tile_pool(name="gath", bufs=2))
        epool = ctx.enter_context(tc.tile_pool(name="esb", bufs=2))
        spool = ctx.enter_context(tc.tile_pool(name="strm", bufs=2))
        hpool = ctx.enter_context(tc.tile_pool(name="hsb", bufs=3))
        tpool = ctx.enter_context(tc.tile_pool(name="tsb", bufs=2))
        mpool = ctx.enter_context(tc.tile_pool(name="tmp", bufs=3))
        psA = ctx.enter_context(tc.tile_pool(name="psA", bufs=4,
                                             space="PSUM"))
        psP = ctx.enter_context(tc.tile_pool(name="psP", bufs=2,
                                             space="PSUM"))
        psZ = ctx.enter_context(tc.tile_pool(name="psZ", bufs=2,
                                             space="PSUM"))

        xT_sb = const.tile([128, NPAD], f32)
        nc.sync.dma_start(out=xT_sb[:], in_=xT_e[:])
        idx_sb = const.tile([128, NCHUNK * 8], i16)
        nc.sync.dma_start(out=idx_sb[:], in_=idx_e[:])
        dinvc_sb = const.tile([128, NBLK], f32)
        nc.sync.dma_start(out=dinvc_sb[:], in_=dinvc_e[:])
        sd_sb = const.tile([128, NBLK * 128], f16)
        nc.sync.dma_start(out=sd_sb[:], in_=sd_e[:])
        qdall_sb = const.tile([128, NPAD], f32)
        nc.sync.dma_start(out=qdall_sb[:], in_=qdall_e[:])
        W_sb = []
        for i in range(3):
            w = const.tile([128, 128], f32, name=f"w{i}_sb")
            nc.sync.dma_start(out=w[:], in_=W_e[i][:])
            W_sb.append(w)
        # f16 copy of W1 (lhsT of the per-block W1-apply matmul)
        w1h_sb = const.tile([128, 128], f16, name="w1h_sb")
        nc.scalar.activation(out=w1h_sb[:], in_=W_sb[0][:],
                             func=mybir.ActivationFunctionType.Copy)
        bias_sb = []
        for i in range(2):
            bcol = const.tile([128, 1], f32, name=f"b{i}_sb")
            nc.sync.dma_start(out=bcol[:], in_=b_e[i][:])
            bias_sb.append(bcol)
        parts_sb = const.tile([128, NBLK], f32)
        nc.vector.memset(parts_sb[:], 0.0)

        # NOTE: gather tables are Local (not Shared) and exactly sized with
        # zero AP offset — dma_gather's Q7 descriptor generator crashes the
        # device on Shared-scratchpad or offset-view sources.
        slab_lo = dram.tile([cfg.LO_PER, C], f16)
        t_lo_full = dram.tile([SPLIT, C], f16, name="t_lo_l2")
        if cfg.HI_PER:
            slab_hi = dram.tile([cfg.HI_PER, C], f16)
            t_hi_full = dram.tile([N - SPLIT, C], f16, name="t_hi_l2")

        rg = [list(range(M))]
        tsb_keep = {}

        def t_block(b, lhsT_ap, w_sb, write_slab):
            """t_blk = dinv * (h_blk @ W); keep in SBUF, optionally to slab."""
            z_ps = psZ.tile([128, 128], f32, tag="zps")
            nc.tensor.matmul(out=z_ps[:], lhsT=lhsT_ap, rhs=w_sb[:],
                             start=True, stop=True)
            t_sb = tpool.tile([128, 128], f16, tag=f"tk{b}")
            nc.scalar.activation(out=t_sb[:], in_=z_ps[:],
                                 func=mybir.ActivationFunctionType.Copy,
                                 scale=dinvc_sb[:, b:b + 1])
            tsb_keep[b] = t_sb
            if write_slab:
                w = blk_width(b)
                which, row = blk_slab_row(b)
                slab = slab_lo if which == 0 else slab_hi
                nc.sync.dma_start(out=slab[row:row + w, :], in_=t_sb[:w, :])

        def emit_allgather(which):
            if which == 0:
                nc.gpsimd.collective_compute(
                    "AllGather", mybir.AluOpType.bypass, replica_groups=rg,
                    ins=[slab_lo[:]], outs=[t_lo_full[:]])
            elif cfg.HI_PER:
                nc.gpsimd.collective_compute(
                    "AllGather", mybir.AluOpType.bypass, replica_groups=rg,
                    ins=[slab_hi[:]], outs=[t_hi_full[:]])

        # ---------------- layer 1: x-space scatter, no gather -----------
        # t1 blocks for self-loops (no slab write; no AllGather of t1)
        for b in range(NBLK):
            t_block(b, xT_sb[:, b * 128:(b + 1) * 128], W_sb[0],
                    write_slab=False)

        ci1 = 0
        for gi, grp in enumerate(groups1):
            nck = sum(C1[b] for b in grp)
            pchain = {}
            if nck > 0:
                c0 = ci1
                xg_sb = spool.tile([128, nck * 128], f16, tag="xg")
                nc.sync.dma_start(
                    out=xg_sb[:], in_=xg_e[:, c0 * 128:(c0 + nck) * 128])
                e1_sb = epool.tile([128, nck * 128], f16, tag="e1")
                nc.sync.dma_start(
                    out=e1_sb[:], in_=e1all_e[:, c0 * 128:(c0 + nck) * 128])
                # per-block P chains in x-space
                for j in range(nck):
                    b = schedule1[c0 + j]
                    if b not in pchain:
                        pchain[b] = psP.tile([128, 128], f32, tag="pps",
                                             name=f"p_b{b}")
                    nc.tensor.matmul(
                        out=pchain[b][:],
                        lhsT=xg_sb[:, j * 128:(j + 1) * 128],
                        rhs=e1_sb[:, j * 128:(j + 1) * 128],
                        start=(j == 0 or schedule1[c0 + j - 1] != b),
                        stop=(j == nck - 1 or schedule1[c0 + j + 1] != b))
                ci1 += nck
            for b in grp:
                # aggT_b = (diag(dinv) t1_b).T + W1^T P_b
                w = blk_width(b)
                agg = psA.tile([128, 128], f32, tag="agg", name=f"agg1_{b}")
                has_p = C1[b] > 0
                nc.tensor.matmul(
                    out=agg[:], lhsT=tsb_keep[b][:w, :],
                    rhs=sd_sb[:w, b * 128:b * 128 + 128],
                    start=True, stop=not has_p)
                if has_p:
                    pc_sb = mpool.tile([128, 128], f16, tag="pc")
                    nc.scalar.activation(
                        out=pc_sb[:], in_=pchain[b][:],
                        func=mybir.ActivationFunctionType.Copy)
                    nc.tensor.matmul(
                        out=agg[:], lhsT=w1h_sb[:], rhs=pc_sb[:],
                        start=False, stop=True)
                # epilogue: h1 = relu(agg + b1); t2 slab
                h_sb = hpool.tile([128, 128], f32, tag="hsb")
                nc.scalar.activation(
                    out=h_sb[:], in_=agg[:],
                    func=mybir.ActivationFunctionType.Relu,
                    bias=bias_sb[0][:])
                t_block(b, h_sb[:], W_sb[1], write_slab=True)
            if gi == last_lo_grp1:
                emit_allgather(0)
            if gi == last_grp1:
                emit_allgather(1)

        # ---------------- layer 2: gather + scatter -----------------
        chain_pos = [0] * NBLK
        ci = 0
        for gi, grp in enumerate(groups):
            agg_tiles = {}
            for b in grp:
                w = blk_width(b)
                agg_tiles[b] = psA.tile([128, 128], f32, tag="agg",
                                        name=f"agg2_{b}")
                nc.tensor.matmul(
                    out=agg_tiles[b][:], lhsT=tsb_keep[b][:w, :],
                    rhs=sd_sb[:w, b * 128:b * 128 + 128],
                    start=True, stop=(total_chunks[b] == 0))
            n_lo = sum(CL[b] for b in grp)
            n_hi = sum(CH[b] for b in grp)
            for half, nck in ((0, n_lo), (1, n_hi)):
                if nck == 0:
                    continue
                c0 = ci
                gt = gpool.tile([128, nck, 128], f16, tag=f"g{half}")
                src_view = t_lo_full[:] if half == 0 else t_hi_full[:]
                if SINGLE_PACKET:
                    # single-packet mode fails on HW above 1024 idxs/call
                    for p0 in range(0, nck, 8):
                        p1 = min(p0 + 8, nck)
                        nc.gpsimd.dma_gather(
                            gt[:, p0:p1, :], src_view,
                            idx_sb[:, (c0 + p0) * 8:(c0 + p1) * 8],
                            num_idxs=(p1 - p0) * 128,
                            num_idxs_reg=(p1 - p0) * 128,
                            elem_size=C, single_packet=True)
                else:
                    nc.gpsimd.dma_gather(
                        gt[:], src_view,
                        idx_sb[:, c0 * 8:(c0 + nck) * 8],
                        num_idxs=nck * 128, num_idxs_reg=nck * 128,
                        elem_size=C, single_packet=False)
                e_sb = epool.tile([128, nck * 128], f16, tag=f"e{half}")
                nc.sync.dma_start(
                    out=e_sb[:],
                    in_=eall_e[:, c0 * 128:(c0 + nck) * 128])
                for j in range(nck):
                    b, h = schedule[c0 + j]
                    nc.tensor.matmul(
                        out=agg_tiles[b][:],
                        lhsT=gt[:, j, :],
                        rhs=e_sb[:, j * 128:(j + 1) * 128],
                        start=False,
                        stop=(chain_pos[b] == total_chunks[b] - 1))
                    chain_pos[b] += 1
                ci += nck
            # epilogue: h2 = relu(agg + b2); weighted reduce with q
            for b in grp:
                h_sb = hpool.tile([128, 128], f32, tag="hsb")
                nc.scalar.activation(
                    out=h_sb[:], in_=agg_tiles[b][:],
                    func=mybir.ActivationFunctionType.Relu,
                    bias=bias_sb[1][:])
                qh = mpool.tile([128, 128], f32, tag="qh")
                nc.vector.tensor_tensor(
                    out=qh[:], in0=h_sb[:],
                    in1=qdall_sb[:, b * 128:(b + 1) * 128],
                    op=mybir.AluOpType.mult)
                nc.vector.reduce_sum(
                    out=parts_sb[:, b:b + 1], in_=qh[:],
                    axis=mybir.AxisListType.X)

        # ---------------- layer 3 (collapsed): v @ W3 ----------------
        v_sb = const.tile([128, 1], f32, name="v_sb")
        nc.vector.reduce_sum(out=v_sb[:], in_=parts_sb[:],
                             axis=mybir.AxisListType.X)
        o_ps = psZ.tile([1, 128], f32, tag="zps")
        nc.tensor.matmul(out=o_ps[:], lhsT=v_sb[:], rhs=W_sb[2][:],
                         start=True, stop=True)
        o_sb = const.tile([1, 128], f32, name="o_sb")
        nc.scalar.activation(out=o_sb[:], in_=o_ps[:],
                             func=mybir.ActivationFunctionType.Copy)
        nc.sync.dma_start(out=out_e[:], in_=o_sb[:])

    nc.compile()
    return nc


# small helper so build_program can use `with TileCtx(...)`
def TileCtx(nc, tile_mod):
    return tile_mod.TileContext(nc)


def run(cfg, meta, per_core, weights, trace=False):
    from concourse.bass_utils import run_bass_kernel_spmd

    nc = build_program(cfg, meta)
    in_maps = []
    for r in range(cfg.M):
        m = dict(per_core[r])
        m["W1"], m["W2"], m["W3"] = weights["W1"], weights["W2"], weights["W3"]
        m["b1"] = weights["b1"].reshape(cfg.C, 1).astype(np.float32)
        m["b2"] = weights["b2"].reshape(cfg.C, 1).astype(np.float32)
        in_maps.append(m)
    res = run_bass_kernel_spmd(nc, in_maps, core_ids=list(range(cfg.M)),
                               trace=trace)
    return res


def kernel(**inputs):
    cfg = GCNConfig()
    x = np.asarray(inputs["x"], dtype=np.float32)
    meta, per_core, dinv = host_prep(cfg, x, inputs["edge_index"])
    weights = {k: np.asarray(inputs[k], dtype=np.float32)
               for k in ("W1", "b1", "W2", "b2", "W3", "b3")}
    res = run(cfg, meta, per_core, weights, trace=False)
    total = np.zeros(cfg.C, dtype=np.float64)
    for r in range(cfg.M):
        total += res.results[r]["out_vec"].astype(np.float64).reshape(-1)
    out = total / cfg.N + weights["b3"].astype(np.float64)
    return out.astype(np.float32)


# revision 23
# speedup vs baseline: 1.0620x; 1.0620x over previous
"""GCN (3-layer, PyG-style symmetric norm) on 8 Trainium2 NeuronCores.

Strategy (hardcoded for N=50000, E=800000, C=128, 8 cores):
  - Nodes sharded by contiguous ranges of 6250 across 8 cores; edges
    partitioned by dst so segment-sum is local to the dst owner.
  - Aggregation per dst block b is a chain of TensorE scatter matmuls:
    aggT_b[ch, col] += G_chunk[slot, ch]^T E_chunk[slot, col], where
    E (host-precomputed, streamed from HBM) carries the edge norms.
  - Layer 1 needs no gather/AllGather: sources are rows of the input x,
    so the host pre-duplicates x rows into chunk-slot order (xg) and
    the scatter runs in input space; W1 is applied per block afterwards
    (linearity), self-loops via a diag matmul on t1 = dinv*(x@W1).
  - Layer 2 computes the table slab t2 = dinv*(h1 @ W2), AllGathers it,
    and gathers source rows with dma_gather (int16 idx; lo/hi halves).
  - Layer 3 collapses algebraically: the final output is a mean over
    all nodes, so out = (sum_n q_n h2[n]) @ W3 / N + b3 with
    host-computed q_n = dinv_n^2 (dinv_n + sum_{e:src=n} dinv_dst_e).
    Only a weighted column-reduce of h2 plus one tiny W3 matmul.
"""

import sys

for _p in ("/opt/trn_rl_repo", "/root/.axon_site/_ro/trn_rl_repo"):
    if _p not in sys.path:
        sys.path.insert(0, _p)

import numpy as np


class GCNConfig:
    """Node ownership: core r owns lo-range [r*LO_PER, (r+1)*LO_PER) and
    hi-range [SPLIT + r*HI_PER, SPLIT + (r+1)*HI_PER).  SPLIT = M*LO_PER
    keeps both gather tables int16-addressable and offset-free."""

    def __init__(self, n_nodes=50000, n_edges=800000, n_cores=8,
                 lo_per_core=4096, blocks_per_group=4, blocks_per_group1=2):
        assert n_nodes % n_cores == 0
        self.N = n_nodes
        self.E = n_edges
        self.C = 128
        self.M = n_cores
        self.LN = n_nodes // n_cores          # local nodes per core
        self.LO_PER = min(lo_per_core, self.LN)
        self.SPLIT = self.LO_PER * n_cores
        self.HI_PER = self.LN - self.LO_PER
        assert self.LO_PER <= 32768 and self.HI_PER <= 32767
        self.NBLK_LO = -(-self.LO_PER // 128)
        self.NBLK_HI = -(-self.HI_PER // 128) if self.HI_PER else 0
        self.NBLK = self.NBLK_LO + self.NBLK_HI
        self.NPAD = self.NBLK * 128           # padded local node count
        self.GB = blocks_per_group            # blocks per gather group (L2)
        self.GB1 = blocks_per_group1          # blocks per stream group (L1)
        self.LO_PIECES = 4                    # pipelined AllGather pieces
        assert self.LO_PER % (128 * self.LO_PIECES) == 0
        assert self.HI_PER == 0 or self.LO_PER % 128 == 0

    def storage_range(self, r):
        """Storage positions of core r's local ordering [0, LN)."""
        g = np.empty(self.LN, dtype=np.int64)
        g[:self.LO_PER] = r * self.LO_PER + np.arange(self.LO_PER)
        if self.HI_PER:
            g[self.LO_PER:] = (self.SPLIT + r * self.HI_PER
                               + np.arange(self.HI_PER))
        return g


def _balance_positions(cfg, w_node):
    """Assign nodes to storage positions, balancing per-(core, block) edge
    loads within each half. Returns pos[node] -> storage position."""
    import heapq
    N, M, SPLIT = cfg.N, cfg.M, cfg.SPLIT
    pos = np.empty(N, dtype=np.int64)
    for half in (0, 1):
        if half == 0:
            ids = np.arange(0, SPLIT)
            nblk, per = cfg.NBLK_LO, cfg.LO_PER
            base = 0
        else:
            if cfg.HI_PER == 0:
                break
            ids = np.arange(SPLIT, N)
            nblk, per = cfg.NBLK_HI, cfg.HI_PER
            base = SPLIT
        bins = []
        cap = {}
        fill = {}
        for r in range(M):
            for b in range(nblk):
                w = min(128, per - b * 128)
                bins.append((0.0, (r, b)))
                cap[(r, b)] = w
                fill[(r, b)] = []
        heapq.heapify(bins)
        order = ids[np.argsort(-w_node[ids], kind="stable")]
        for n in order:
            while True:
                load, key = heapq.heappop(bins)
                if len(fill[key]) < cap[key]:
                    break
            fill[key].append(n)
            if len(fill[key]) < cap[key]:
                heapq.heappush(bins, (load + float(w_node[n]), key))
        for (r, b), members in fill.items():
            start = base + r * per + b * 128
            for i, n in enumerate(members):
                pos[n] = start + i
    return pos


def host_prep(cfg, x, edge_index, dinv=None):
    """Build per-core input arrays + the shared chunk schedules."""
    N, M, LN, NBLK, SPLIT = cfg.N, cfg.M, cfg.LN, cfg.NBLK, cfg.SPLIT

    src0 = np.asarray(edge_index[0], dtype=np.int64)
    dst0 = np.asarray(edge_index[1], dtype=np.int64)

    if dinv is None:
        deg = (np.bincount(dst0, minlength=N) + 1).astype(np.float32)
        dinv = (1.0 / np.sqrt(deg)).astype(np.float32)

    # balance (core, block) bin loads; nodes keep their half
    w_node = np.bincount(dst0, minlength=N).astype(np.float64)
    pos = _balance_positions(cfg, w_node)
    inv = np.empty(N, dtype=np.int64)
    inv[pos] = np.arange(N)
    dinv_stor = dinv[inv]                     # dinv by storage position

    src_all = pos[src0]
    dst_all = pos[dst0]

    # dst position -> (owner core, local index) under the lo/hi ownership
    is_hi_dst = dst_all >= SPLIT
    q = dst_all - SPLIT
    core = np.where(is_hi_dst, q // max(cfg.HI_PER, 1), dst_all // cfg.LO_PER)
    li = np.where(is_hi_dst, cfg.LO_PER + q % max(cfg.HI_PER, 1),
                  dst_all % cfg.LO_PER)
    blk = li // 128
    dloc = li % 128
    half = (src_all >= SPLIT).astype(np.int64)

    # ---- layer-2 schedule: (core, block, half) runs
    key = (core * NBLK + blk) * 2 + half
    order = np.argsort(key, kind="stable")
    s_src = src_all[order]
    s_dloc = dloc[order]
    counts = np.bincount(key, minlength=M * NBLK * 2).reshape(M, NBLK, 2)
    starts = np.zeros(M * NBLK * 2 + 1, dtype=np.int64)
    np.cumsum(counts.reshape(-1), out=starts[1:])

    CL = ((counts[:, :, 0] + 127) // 128).max(axis=0)
    CH = ((counts[:, :, 1] + 127) // 128).max(axis=0)

    groups = [list(range(g, min(g + cfg.GB, NBLK)))
              for g in range(0, NBLK, cfg.GB)]
    schedule = []  # (block, half)
    for grp in groups:
        for b in grp:
            schedule += [(b, 0)] * int(CL[b])
        for b in grp:
            schedule += [(b, 1)] * int(CH[b])
    NCHUNK = len(schedule)

    # ---- layer-1 schedule: (core, block) runs (halves merged; no gather)
    key1 = core * NBLK + blk
    order1 = np.argsort(key1, kind="stable")
    s1_src = src_all[order1]                  # global storage position
    s1_dloc = dloc[order1]
    counts1 = counts.sum(axis=2)              # [M, NBLK]
    starts1 = np.zeros(M * NBLK + 1, dtype=np.int64)
    np.cumsum(counts1.reshape(-1), out=starts1[1:])
    C1 = ((counts1 + 127) // 128).max(axis=0)

    groups1 = [list(range(g, min(g + cfg.GB1, NBLK)))
               for g in range(0, NBLK, cfg.GB1)]
    schedule1 = []  # block
    for grp in groups1:
        for b in grp:
            schedule1 += [b] * int(C1[b])
    NCHUNK1 = len(schedule1)

    # x by storage position (for layer-1 host pre-gather)
    x_stor = np.zeros((SPLIT + max(N - SPLIT, 0), 128), dtype=np.float32)
    x_stor[pos] = np.asarray(x, dtype=np.float32)

    # layer-3 collapse coefficients: q_n = dinv_n (dinv_n + sum dinv_dst)
    csum = np.bincount(src0, weights=dinv[dst0].astype(np.float64),
                       minlength=N)
    q_node = (dinv.astype(np.float64)
              * (dinv.astype(np.float64) + csum)).astype(np.float32)
    q_stor = q_node[inv]

    cols = np.arange(128, dtype=np.float32)
    per_core = []
    for r in range(M):
        # ---- layer-2 idx + E arrays
        idx_arr = np.zeros((NCHUNK, 128), dtype=np.int64)
        dl_arr = np.full((NCHUNK, 128), -1.0, dtype=np.float32)
        pos_in = {}
        for ci, (b, h) in enumerate(schedule):
            k = pos_in.get((b, h), 0)
            pos_in[(b, h)] = k + 1
            kk = (r * NBLK + b) * 2 + h
            lo, hi = starts[kk], starts[kk + 1]
            a = lo + k * 128
            nreal = max(0, min(128, hi - a))
            if nreal > 0:
                seg = slice(a, a + nreal)
                sv = s_src[seg]
                if cfg.LO_PIECES > 1:
                    # lo table is piece-major: [piece, core, row-in-piece]
                    pc = cfg.LO_PER // cfg.LO_PIECES
                    rr, pp = sv // cfg.LO_PER, sv % cfg.LO_PER
                    lo_remap = ((pp // pc) * (pc * M) + rr * pc + pp % pc)
                    idx_arr[ci, :nreal] = np.where(
                        sv < SPLIT, lo_remap, sv - SPLIT)
                else:
                    idx_arr[ci, :nreal] = sv - (SPLIT if h else 0)
                dl_arr[ci, :nreal] = s_dloc[seg]
        flat = idx_arr.reshape(-1)
        w16 = flat.reshape(-1, 16).T.astype(np.int16)  # [16, NCHUNK*8]
        idxw = np.tile(w16, (8, 1))                    # [128, NCHUNK*8]

        g = inv[cfg.storage_range(r)]
        dv = np.zeros(cfg.NPAD, dtype=np.float32)
        dv[:LN] = dinv[g]
        dinvc = np.ascontiguousarray(dv.reshape(NBLK, 128).T)  # [128, NBLK]

        # layer-2 E: E[slot, j] = dinv_dst  (src dinv lives in the table)
        blks = np.array([b for (b, h) in schedule])
        dvblk = dinvc.T[blks]                        # [NCHUNK, 128]
        E = (dl_arr[:, :, None] == cols[None, None, :]).astype(np.float16)
        E *= dvblk[:, None, :].astype(np.float16)
        eall = np.ascontiguousarray(
            E.transpose(1, 0, 2).reshape(128, NCHUNK * 128))

        # ---- layer-1 xg + E1 arrays (chunk-slot order, x rows duplicated)
        src1_arr = np.zeros((NCHUNK1, 128), dtype=np.int64)
        dl1_arr = np.full((NCHUNK1, 128), -1.0, dtype=np.float32)
        sc1_arr = np.zeros((NCHUNK1, 128), dtype=np.float32)  # dinv_src
        pos_in1 = {}
        for ci, b in enumerate(schedule1):
            k = pos_in1.get(b, 0)
            pos_in1[b] = k + 1
            kk = r * NBLK + b
            lo, hi = starts1[kk], starts1[kk + 1]
            a = lo + k * 128
            nreal = max(0, min(128, hi - a))
            if nreal > 0:
                seg = slice(a, a + nreal)
                src1_arr[ci, :nreal] = s1_src[seg]
                dl1_arr[ci, :nreal] = s1_dloc[seg]
                sc1_arr[ci, :nreal] = dinv_stor[s1_src[seg]]
        # xg: [slot, chunk, ch] -> [128, NCHUNK1*128] (pure row duplication)
        xg = x_stor[src1_arr.reshape(-1)].astype(np.float16)
        xg = np.ascontiguousarray(
            xg.reshape(NCHUNK1, 128, 128).transpose(1, 0, 2)
            .reshape(128, NCHUNK1 * 128))
        # E1: full norm dinv_src * dinv_dst
        blks1 = np.array(schedule1)
        dvblk1 = dinvc.T[blks1]                      # [NCHUNK1, 128]
        E1 = (dl1_arr[:, :, None] == cols[None, None, :]).astype(np.float32)
        E1 *= sc1_arr[:, :, None] * dvblk1[:, None, :]
        e1all = np.ascontiguousarray(
            E1.astype(np.float16).transpose(1, 0, 2)
            .reshape(128, NCHUNK1 * 128))

        # self-loop diag(dinv) per block
        sd = np.zeros((128, NBLK * 128), dtype=np.float16)
        for b in range(NBLK):
            np.fill_diagonal(sd[:, b * 128:(b + 1) * 128],
                             dinvc[:, b].astype(np.float16))

        # layer-3 weighted-reduce coefficients, broadcast across partitions
        qv = np.zeros(cfg.NPAD, dtype=np.float32)
        qv[:LN] = q_stor[cfg.storage_range(r)]
        qdall = np.ascontiguousarray(
            np.broadcast_to(qv[None, :], (128, cfg.NPAD)))

        xs = x_stor[cfg.storage_range(r)]
        xT = np.zeros((128, cfg.NPAD), dtype=np.float32)
        xT[:, :LN] = xs.T

        per_core.append(dict(xT=xT, idxw=idxw, dinvc=dinvc, eall=eall,
                             xg=xg, e1all=e1all, sd=sd, qdall=qdall))

    meta = dict(CL=CL.astype(int).tolist(), CH=CH.astype(int).tolist(),
                C1=C1.astype(int).tolist(), groups=groups, groups1=groups1,
                schedule=schedule, schedule1=schedule1,
                NCHUNK=NCHUNK, NCHUNK1=NCHUNK1)
    return meta, per_core, dinv


SINGLE_PACKET = False


def build_program(cfg, meta):
    import concourse.bass as bass
    import concourse.bacc as bacc
    import concourse.tile as tile
    from concourse import mybir
    from contextlib import ExitStack

    f32 = mybir.dt.float32
    f16 = mybir.dt.float16
    i16 = mybir.dt.int16
    N, C, M, LN = cfg.N, cfg.C, cfg.M, cfg.LN
    NBLK, NPAD, SPLIT = cfg.NBLK, cfg.NPAD, cfg.SPLIT

    def blk_width(b):
        if b < cfg.NBLK_LO:
            return min(128, cfg.LO_PER - b * 128)
        return min(128, cfg.HI_PER - (b - cfg.NBLK_LO) * 128)

    def blk_slab_row(b):
        if b < cfg.NBLK_LO:
            return 0, b * 128
        return 1, (b - cfg.NBLK_LO) * 128

    CL, CH, C1 = meta["CL"], meta["CH"], meta["C1"]
    groups, groups1 = meta["groups"], meta["groups1"]
    schedule, schedule1 = meta["schedule"], meta["schedule1"]
    NCHUNK, NCHUNK1 = meta["NCHUNK"], meta["NCHUNK1"]

    total_chunks = [CL[b] + CH[b] for b in range(NBLK)]
    last_grp1 = len(groups1) - 1

    nc = bacc.Bacc(None, target_bir_lowering=False, debug=False)
    xT_e = nc.declare_dram_parameter("xT", [128, NPAD], f32, isOutput=False)
    idx_e = nc.declare_dram_parameter("idxw", [128, NCHUNK * 8], i16,
                                      isOutput=False)
    dinvc_e = nc.declare_dram_parameter("dinvc", [128, NBLK], f32,
                                        isOutput=False)
    eall_e = nc.declare_dram_parameter("eall", [128, NCHUNK * 128], f16,
                                       isOutput=False)
    xg_e = nc.declare_dram_parameter("xg", [128, NCHUNK1 * 128], f16,
                                     isOutput=False)
    e1all_e = nc.declare_dram_parameter("e1all", [128, NCHUNK1 * 128], f16,
                                        isOutput=False)
    sd_e = nc.declare_dram_parameter("sd", [128, NBLK * 128], f16,
                                     isOutput=False)
    qdall_e = nc.declare_dram_parameter("qdall", [128, NPAD], f32,
                                        isOutput=False)
    W_e = [nc.declare_dram_parameter(f"W{i}", [C, C], f32, isOutput=False)
           for i in (1, 2, 3)]
    b_e = [nc.declare_dram_parameter(f"b{i}", [C, 1], f32, isOutput=False)
           for i in (1, 2)]
    out_e = nc.declare_dram_parameter("out_vec", [1, C], f32, isOutput=True)

    with TileCtx(nc, tile) as tc, ExitStack() as ctx:
        const = ctx.enter_context(tc.tile_pool(name="const", bufs=1))
        dram = ctx.enter_context(tc.tile_pool(name="dram", bufs=1,
                                              space="DRAM"))
        gpool = ctx.enter_context(tc.tile_pool(name="gath", bufs=2))
        epool = ctx.enter_context(tc.tile_pool(name="esb", bufs=2))
        spool = ctx.enter_context(tc.tile_pool(name="strm", bufs=2))
        hpool = ctx.enter_context(tc.tile_pool(name="hsb", bufs=3))
        tpool = ctx.enter_context(tc.tile_pool(name="tsb", bufs=2))
        mpool = ctx.enter_context(tc.tile_pool(name="tmp", bufs=3))
        psA = ctx.enter_context(tc.tile_pool(name="psA", bufs=4,
                                             space="PSUM"))
        psP = ctx.enter_context(tc.tile_pool(name="psP", bufs=2,
                                             space="PSUM"))
        psZ = ctx.enter_context(tc.tile_pool(name="psZ", bufs=2,
                                             space="PSUM"))

        xT_sb = const.tile([128, NPAD], f32)
        nc.sync.dma_start(out=xT_sb[:], in_=xT_e[:])
        idx_sb = const.tile([128, NCHUNK * 8], i16)
        nc.sync.dma_start(out=idx_sb[:], in_=idx_e[:])
        dinvc_sb = const.tile([128, NBLK], f32)
        nc.sync.dma_start(out=dinvc_sb[:], in_=dinvc_e[:])
        sd_sb = const.tile([128, NBLK * 128], f16)
        nc.sync.dma_start(out=sd_sb[:], in_=sd_e[:])
        qdall_sb = const.tile([128, NPAD], f32)
        nc.sync.dma_start(out=qdall_sb[:], in_=qdall_e[:])
        W_sb = []
        for i in range(3):
            w = const.tile([128, 128], f32, name=f"w{i}_sb")
            nc.sync.dma_start(out=w[:], in_=W_e[i][:])
            W_sb.append(w)
        # f16 copy of W1 (lhsT of the per-block W1-apply matmul)
        w1h_sb = const.tile([128, 128], f16, name="w1h_sb")
        nc.scalar.activation(out=w1h_sb[:], in_=W_sb[0][:],
                             func=mybir.ActivationFunctionType.Copy)
        bias_sb = []
        for i in range(2):
            bcol = const.tile([128, 1], f32, name=f"b{i}_sb")
            nc.sync.dma_start(out=bcol[:], in_=b_e[i][:])
            bias_sb.append(bcol)
        parts_sb = const.tile([128, NBLK], f32)
        nc.vector.memset(parts_sb[:], 0.0)

        # NOTE: gather tables are Local (not Shared) and exactly sized with
        # zero AP offset — dma_gather's Q7 descriptor generator crashes the
        # device on Shared-scratchpad or offset-view sources.
        slab_lo = dram.tile([cfg.LO_PER, C], f16)
        t_lo_full = dram.tile([SPLIT, C], f16, name="t_lo_l2")
        if cfg.HI_PER:
            slab_hi = dram.tile([cfg.HI_PER, C], f16)
            t_hi_full = dram.tile([N - SPLIT, C], f16, name="t_hi_l2")

        rg = [list(range(M))]
        tsb_keep = {}

        def t_block(b, lhsT_ap, w_sb, write_slab):
            """t_blk = dinv * (h_blk @ W); keep in SBUF, optionally to slab."""
            z_ps = psZ.tile([128, 128], f32, tag="zps")
            nc.tensor.matmul(out=z_ps[:], lhsT=lhsT_ap, rhs=w_sb[:],
                             start=True, stop=True)
            t_sb = tpool.tile([128, 128], f16, tag=f"tk{b}")
            nc.scalar.activation(out=t_sb[:], in_=z_ps[:],
                                 func=mybir.ActivationFunctionType.Copy,
                                 scale=dinvc_sb[:, b:b + 1])
            tsb_keep[b] = t_sb
            if write_slab:
                w = blk_width(b)
                which, row = blk_slab_row(b)
                slab = slab_lo if which == 0 else slab_hi
                nc.sync.dma_start(out=slab[row:row + w, :], in_=t_sb[:w, :])

        def emit_allgather(which, piece=None):
            if which == 0:
                # piece-major lo table: piece k holds [8 cores x pc rows]
                pc = cfg.LO_PER // cfg.LO_PIECES
                k = piece
                nc.gpsimd.collective_compute(
                    "AllGather", mybir.AluOpType.bypass, replica_groups=rg,
                    ins=[slab_lo[k * pc:(k + 1) * pc, :]],
                    outs=[t_lo_full[k * pc * M:(k + 1) * pc * M, :]])
            elif cfg.HI_PER:
                nc.gpsimd.collective_compute(
                    "AllGather", mybir.AluOpType.bypass, replica_groups=rg,
                    ins=[slab_hi[:]], outs=[t_hi_full[:]])

        # ---------------- layer 1: x-space scatter, no gather -----------
        # t1 blocks for self-loops (no slab write; no AllGather of t1)
        for b in range(NBLK):
            t_block(b, xT_sb[:, b * 128:(b + 1) * 128], W_sb[0],
                    write_slab=False)

        ci1 = 0
        for gi, grp in enumerate(groups1):
            nck = sum(C1[b] for b in grp)
            pchain = {}
            if nck > 0:
                c0 = ci1
                xg_sb = spool.tile([128, nck * 128], f16, tag="xg")
                nc.sync.dma_start(
                    out=xg_sb[:], in_=xg_e[:, c0 * 128:(c0 + nck) * 128])
                e1_sb = epool.tile([128, nck * 128], f16, tag="e1")
                nc.sync.dma_start(
                    out=e1_sb[:], in_=e1all_e[:, c0 * 128:(c0 + nck) * 128])
                # per-block P chains in x-space
                for j in range(nck):
                    b = schedule1[c0 + j]
                    if b not in pchain:
                        pchain[b] = psP.tile([128, 128], f32, tag="pps",
                                             name=f"p_b{b}")
                    nc.tensor.matmul(
                        out=pchain[b][:],
                        lhsT=xg_sb[:, j * 128:(j + 1) * 128],
                        rhs=e1_sb[:, j * 128:(j + 1) * 128],
                        start=(j == 0 or schedule1[c0 + j - 1] != b),
                        stop=(j == nck - 1 or schedule1[c0 + j + 1] != b))
                ci1 += nck
            for b in grp:
                # aggT_b = (diag(dinv) t1_b).T + W1^T P_b
                w = blk_width(b)
                agg = psA.tile([128, 128], f32, tag="agg", name=f"agg1_{b}")
                has_p = C1[b] > 0
                nc.tensor.matmul(
                    out=agg[:], lhsT=tsb_keep[b][:w, :],
                    rhs=sd_sb[:w, b * 128:b * 128 + 128],
                    start=True, stop=not has_p)
                if has_p:
                    pc_sb = mpool.tile([128, 128], f16, tag="pc")
                    nc.scalar.activation(
                        out=pc_sb[:], in_=pchain[b][:],
                        func=mybir.ActivationFunctionType.Copy)
                    nc.tensor.matmul(
                        out=agg[:], lhsT=w1h_sb[:], rhs=pc_sb[:],
                        start=False, stop=True)
                # epilogue: h1 = relu(agg + b1); t2 slab
                h_sb = hpool.tile([128, 128], f32, tag="hsb")
                nc.scalar.activation(
                    out=h_sb[:], in_=agg[:],
                    func=mybir.ActivationFunctionType.Relu,
                    bias=bias_sb[0][:])
                t_block(b, h_sb[:], W_sb[1], write_slab=True)
            bpp = cfg.LO_PER // cfg.LO_PIECES // 128  # blocks per lo piece
            for k in range(cfg.LO_PIECES):
                if grp[-1] == (k + 1) * bpp - 1:
                    emit_allgather(0, piece=k)
            if gi == last_grp1:
                emit_allgather(1)

        # ---------------- layer 2: gather + scatter -----------------
        chain_pos = [0] * NBLK
        ci = 0
        for gi, grp in enumerate(groups):
            agg_tiles = {}
            for b in grp:
                w = blk_width(b)
                agg_tiles[b] = psA.tile([128, 128], f32, tag="agg",
                                        name=f"agg2_{b}")
                nc.tensor.matmul(
                    out=agg_tiles[b][:], lhsT=tsb_keep[b][:w, :],
                    rhs=sd_sb[:w, b * 128:b * 128 + 128],
                    start=True, stop=(total_chunks[b] == 0))
            n_lo = sum(CL[b] for b in grp)
            n_hi = sum(CH[b] for b in grp)
            for half, nck in ((0, n_lo), (1, n_hi)):
                if nck == 0:
                    continue
                c0 = ci
                c0 = ci
                src_view = t_lo_full[:] if half == 0 else t_hi_full[:]
                e_sb = epool.tile([128, nck * 128], f16, tag=f"e{half}")
                nc.sync.dma_start(
                    out=e_sb[:],
                    in_=eall_e[:, c0 * 128:(c0 + nck) * 128])
                if USE_PREP:
                    # prepare_only + per-piece trigger: the engine pays only
                    # descriptor generation, not DMA-completion wait.  The
                    # rotating per-piece tiles bound prep runahead (WAR) so
                    # the SWDGE descriptor ring (~16K descs) cannot overflow.
                    PIECE = 12
                    for p0 in range(0, nck, PIECE):
                        p1 = min(p0 + PIECE, nck)
                        gtp = gpool.tile([128, PIECE, 128], f16,
                                         tag=f"p{(prep_rot[0]) % 3}")
                        prep_rot[0] += 1
                        nc.gpsimd.dma_gather(
                            gtp[:, :p1 - p0, :], src_view,
                            idx_sb[:, (c0 + p0) * 8:(c0 + p1) * 8],
                            num_idxs=(p1 - p0) * 128,
                            num_idxs_reg=(p1 - p0) * 128,
                            elem_size=C, single_packet=False,
                            prepare_only=True, sem=gsem)
                        nc.gpsimd.trigger_dma(count=None)
                        for j in range(p0, p1):
                            b, h = schedule[c0 + j]
                            nc.tensor.matmul(
                                out=agg_tiles[b][:],
                                lhsT=gtp[:, j - p0, :],
                                rhs=e_sb[:, j * 128:(j + 1) * 128],
                                start=False,
                                stop=(chain_pos[b] == total_chunks[b] - 1))
                            chain_pos[b] += 1
                else:
                    gt = gpool.tile([128, nck, 128], f16, tag=f"g{half}")
                    nc.gpsimd.dma_gather(
                        gt[:], src_view,
                        idx_sb[:, c0 * 8:(c0 + nck) * 8],
                        num_idxs=nck * 128, num_idxs_reg=nck * 128,
                        elem_size=C, single_packet=False)
                    for j in range(nck):
                        b, h = schedule[c0 + j]
                        nc.tensor.matmul(
                            out=agg_tiles[b][:],
                            lhsT=gt[:, j, :],
                            rhs=e_sb[:, j * 128:(j + 1) * 128],
                            start=False,
                            stop=(chain_pos[b] == total_chunks[b] - 1))
                        chain_pos[b] += 1
                ci += nck
            # epilogue: h2 = relu(agg + b2); weighted reduce with q
            for b in grp:
                h_sb = hpool.tile([128, 128], f32, tag="hsb")
                nc.scalar.activation(
                    out=h_sb[:], in_=agg_tiles[b][:],
                    func=mybir.ActivationFunctionType.Relu,
                    bias=bias_sb[1][:])
                qh = mpool.tile([128, 128], f32, tag="qh")
                nc.vector.tensor_tensor(
                    out=qh[:], in0=h_sb[:],
                    in1=qdall_sb[:, b * 128:(b + 1) * 128],
                    op=mybir.AluOpType.mult)
                nc.vector.reduce_sum(
                    out=parts_sb[:, b:b + 1], in_=qh[:],
                    axis=mybir.AxisListType.X)

        # ---------------- layer 3 (collapsed): v @ W3 ----------------
        v_sb = const.tile([128, 1], f32, name="v_sb")
        nc.vector.reduce_sum(out=v_sb[:], in_=parts_sb[:],
                             axis=mybir.AxisListType.X)
        o_ps = psZ.tile([1, 128], f32, tag="zps")
        nc.tensor.matmul(out=o_ps[:], lhsT=v_sb[:], rhs=W_sb[2][:],
                         start=True, stop=True)
        o_sb = const.tile([1, 128], f32, name="o_sb")
        nc.scalar.activation(out=o_sb[:], in_=o_ps[:],
                             func=mybir.ActivationFunctionType.Copy)
        nc.sync.dma_start(out=out_e[:], in_=o_sb[:])

    nc.compile()
    return nc


# small helper so build_program can use `with TileCtx(...)`
def TileCtx(nc, tile_mod):
    return tile_mod.TileContext(nc)


def run(cfg, meta, per_core, weights, trace=False):
    from concourse.bass_utils import run_bass_kernel_spmd

    nc = build_program(cfg, meta)
    in_maps = []
    for r in range(cfg.M):
        m = dict(per_core[r])
        m["W1"], m["W2"], m["W3"] = weights["W1"], weights["W2"], weights["W3"]
        m["b1"] = weights["b1"].reshape(cfg.C, 1).astype(np.float32)
        m["b2"] = weights["b2"].reshape(cfg.C, 1).astype(np.float32)
        in_maps.append(m)
    res = run_bass_kernel_spmd(nc, in_maps, core_ids=list(range(cfg.M)),
                               trace=trace)
    return res


def kernel(**inputs):
    cfg = GCNConfig()
    x = np.asarray(inputs["x"], dtype=np.float32)
    meta, per_core, dinv = host_prep(cfg, x, inputs["edge_index"])
    weights = {k: np.asarray(inputs[k], dtype=np.float32)
               for k in ("W1", "b1", "W2", "b2", "W3", "b3")}
    res = run(cfg, meta, per_core, weights, trace=False)
    total = np.zeros(cfg.C, dtype=np.float64)
    for r in range(cfg.M):
        total += res.results[r]["out_vec"].astype(np.float64).reshape(-1)
    out = total / cfg.N + weights["b3"].astype(np.float64)
    return out.astype(np.float32)


# revision 36
# speedup vs baseline: 1.1309x; 1.0648x over previous
"""GCN (3-layer, PyG-style symmetric norm) on 8 Trainium2 NeuronCores.

Strategy (hardcoded for N=50000, E=800000, C=128, 8 cores):
  - Nodes sharded by contiguous ranges of 6250 across 8 cores; edges
    partitioned by dst so segment-sum is local to the dst owner.
  - Aggregation per dst block b is a chain of TensorE scatter matmuls:
    aggT_b[ch, col] += G_chunk[slot, ch]^T E_chunk[slot, col], where
    E (host-precomputed, streamed from HBM) carries the edge norms.
  - Layer 1 needs no gather/AllGather: sources are rows of the input x,
    so the host pre-duplicates x rows into chunk-slot order (xg) and
    the scatter runs in input space; W1 is applied per block afterwards
    (linearity), self-loops via a diag matmul on t1 = dinv*(x@W1).
  - Layer 2 computes the table slab t2 = dinv*(h1 @ W2), AllGathers it,
    and gathers source rows with dma_gather (int16 idx; lo/hi halves).
  - Layer 3 collapses algebraically: the final output is a mean over
    all nodes, so out = (sum_n q_n h2[n]) @ W3 / N + b3 with
    host-computed q_n = dinv_n^2 (dinv_n + sum_{e:src=n} dinv_dst_e).
    Only a weighted column-reduce of h2 plus one tiny W3 matmul.
"""

import sys

for _p in ("/opt/trn_rl_repo", "/root/.axon_site/_ro/trn_rl_repo"):
    if _p not in sys.path:
        sys.path.insert(0, _p)

import numpy as np


class GCNConfig:
    """Node ownership: core r owns lo-range [r*LO_PER, (r+1)*LO_PER) and
    hi-range [SPLIT + r*HI_PER, SPLIT + (r+1)*HI_PER).  SPLIT = M*LO_PER
    keeps both gather tables int16-addressable and offset-free."""

    def __init__(self, n_nodes=50000, n_edges=800000, n_cores=8,
                 lo_per_core=4096, blocks_per_group=4, blocks_per_group1=2):
        assert n_nodes % n_cores == 0
        self.N = n_nodes
        self.E = n_edges
        self.C = 128
        self.M = n_cores
        self.LN = n_nodes // n_cores          # local nodes per core
        self.LO_PER = min(lo_per_core, self.LN)
        self.SPLIT = self.LO_PER * n_cores
        self.HI_PER = self.LN - self.LO_PER
        assert self.LO_PER <= 32768 and self.HI_PER <= 32767
        self.NBLK_LO = -(-self.LO_PER // 128)
        self.NBLK_HI = -(-self.HI_PER // 128) if self.HI_PER else 0
        self.NBLK = self.NBLK_LO + self.NBLK_HI
        self.NPAD = self.NBLK * 128           # padded local node count
        self.GB = blocks_per_group            # blocks per gather group (L2)
        self.GB1 = blocks_per_group1          # blocks per stream group (L1)
        self.LO_PIECES = 4                    # pipelined AllGather pieces
        assert self.LO_PER % (128 * self.LO_PIECES) == 0
        assert self.HI_PER == 0 or self.LO_PER % 128 == 0

    def storage_range(self, r):
        """Storage positions of core r's local ordering [0, LN)."""
        g = np.empty(self.LN, dtype=np.int64)
        g[:self.LO_PER] = r * self.LO_PER + np.arange(self.LO_PER)
        if self.HI_PER:
            g[self.LO_PER:] = (self.SPLIT + r * self.HI_PER
                               + np.arange(self.HI_PER))
        return g


def _balance_positions(cfg, w_node):
    """Assign nodes to storage positions, balancing per-(core, block) edge
    loads within each half. Returns pos[node] -> storage position."""
    import heapq
    N, M, SPLIT = cfg.N, cfg.M, cfg.SPLIT
    pos = np.empty(N, dtype=np.int64)
    for half in (0, 1):
        if half == 0:
            ids = np.arange(0, SPLIT)
            nblk, per = cfg.NBLK_LO, cfg.LO_PER
            base = 0
        else:
            if cfg.HI_PER == 0:
                break
            ids = np.arange(SPLIT, N)
            nblk, per = cfg.NBLK_HI, cfg.HI_PER
            base = SPLIT
        bins = []
        cap = {}
        fill = {}
        for r in range(M):
            for b in range(nblk):
                w = min(128, per - b * 128)
                bins.append((0.0, (r, b)))
                cap[(r, b)] = w
                fill[(r, b)] = []
        heapq.heapify(bins)
        order = ids[np.argsort(-w_node[ids], kind="stable")]
        for n in order:
            while True:
                load, key = heapq.heappop(bins)
                if len(fill[key]) < cap[key]:
                    break
            fill[key].append(n)
            if len(fill[key]) < cap[key]:
                heapq.heappush(bins, (load + float(w_node[n]), key))
        for (r, b), members in fill.items():
            start = base + r * per + b * 128
            for i, n in enumerate(members):
                pos[n] = start + i
    return pos


def host_prep(cfg, x, edge_index, dinv=None):
    """Build per-core input arrays + the shared chunk schedules."""
    N, M, LN, NBLK, SPLIT = cfg.N, cfg.M, cfg.LN, cfg.NBLK, cfg.SPLIT

    src0 = np.asarray(edge_index[0], dtype=np.int64)
    dst0 = np.asarray(edge_index[1], dtype=np.int64)

    if dinv is None:
        deg = (np.bincount(dst0, minlength=N) + 1).astype(np.float32)
        dinv = (1.0 / np.sqrt(deg)).astype(np.float32)

    # balance (core, block) bin loads; nodes keep their half
    w_node = np.bincount(dst0, minlength=N).astype(np.float64)
    pos = _balance_positions(cfg, w_node)
    inv = np.empty(N, dtype=np.int64)
    inv[pos] = np.arange(N)
    dinv_stor = dinv[inv]                     # dinv by storage position

    src_all = pos[src0]
    dst_all = pos[dst0]

    # dst position -> (owner core, local index) under the lo/hi ownership
    is_hi_dst = dst_all >= SPLIT
    q = dst_all - SPLIT
    core = np.where(is_hi_dst, q // max(cfg.HI_PER, 1), dst_all // cfg.LO_PER)
    li = np.where(is_hi_dst, cfg.LO_PER + q % max(cfg.HI_PER, 1),
                  dst_all % cfg.LO_PER)
    blk = li // 128
    dloc = li % 128
    half = (src_all >= SPLIT).astype(np.int64)

    # ---- layer-2 schedule: (core, block, half) runs
    key = (core * NBLK + blk) * 2 + half
    order = np.argsort(key, kind="stable")
    s_src = src_all[order]
    s_dloc = dloc[order]
    counts = np.bincount(key, minlength=M * NBLK * 2).reshape(M, NBLK, 2)
    starts = np.zeros(M * NBLK * 2 + 1, dtype=np.int64)
    np.cumsum(counts.reshape(-1), out=starts[1:])

    CL = ((counts[:, :, 0] + 127) // 128).max(axis=0)
    CH = ((counts[:, :, 1] + 127) // 128).max(axis=0)

    groups = [list(range(g, min(g + cfg.GB, NBLK)))
              for g in range(0, NBLK, cfg.GB)]
    schedule = []  # (block, half)
    for grp in groups:
        for b in grp:
            schedule += [(b, 0)] * int(CL[b])
        for b in grp:
            schedule += [(b, 1)] * int(CH[b])
    NCHUNK = len(schedule)

    # ---- layer-1 schedule: (core, block) runs (halves merged; no gather)
    key1 = core * NBLK + blk
    order1 = np.argsort(key1, kind="stable")
    s1_src = src_all[order1]                  # global storage position
    s1_dloc = dloc[order1]
    counts1 = counts.sum(axis=2)              # [M, NBLK]
    starts1 = np.zeros(M * NBLK + 1, dtype=np.int64)
    np.cumsum(counts1.reshape(-1), out=starts1[1:])
    C1 = ((counts1 + 127) // 128).max(axis=0)

    groups1 = [list(range(g, min(g + cfg.GB1, NBLK)))
               for g in range(0, NBLK, cfg.GB1)]
    schedule1 = []  # block
    for grp in groups1:
        for b in grp:
            schedule1 += [b] * int(C1[b])
    NCHUNK1 = len(schedule1)

    # x by storage position (for layer-1 host pre-gather)
    x_stor = np.zeros((SPLIT + max(N - SPLIT, 0), 128), dtype=np.float32)
    x_stor[pos] = np.asarray(x, dtype=np.float32)

    # layer-3 collapse coefficients: q_n = dinv_n (dinv_n + sum dinv_dst)
    csum = np.bincount(src0, weights=dinv[dst0].astype(np.float64),
                       minlength=N)
    q_node = (dinv.astype(np.float64)
              * (dinv.astype(np.float64) + csum)).astype(np.float32)
    q_stor = q_node[inv]

    cols = np.arange(128, dtype=np.float32)
    per_core = []
    for r in range(M):
        # ---- layer-2 idx + E arrays
        idx_arr = np.zeros((NCHUNK, 128), dtype=np.int64)
        dl_arr = np.full((NCHUNK, 128), -1.0, dtype=np.float32)
        pos_in = {}
        for ci, (b, h) in enumerate(schedule):
            k = pos_in.get((b, h), 0)
            pos_in[(b, h)] = k + 1
            kk = (r * NBLK + b) * 2 + h
            lo, hi = starts[kk], starts[kk + 1]
            a = lo + k * 128
            nreal = max(0, min(128, hi - a))
            if nreal > 0:
                seg = slice(a, a + nreal)
                sv = s_src[seg]
                if cfg.LO_PIECES > 1:
                    # lo table is piece-major: [piece, core, row-in-piece]
                    pc = cfg.LO_PER // cfg.LO_PIECES
                    rr, pp = sv // cfg.LO_PER, sv % cfg.LO_PER
                    lo_remap = ((pp // pc) * (pc * M) + rr * pc + pp % pc)
                    idx_arr[ci, :nreal] = np.where(
                        sv < SPLIT, lo_remap, sv - SPLIT)
                else:
                    idx_arr[ci, :nreal] = sv - (SPLIT if h else 0)
                dl_arr[ci, :nreal] = s_dloc[seg]
        flat = idx_arr.reshape(-1)
        w16 = flat.reshape(-1, 16).T.astype(np.int16)  # [16, NCHUNK*8]
        idxw = np.tile(w16, (8, 1))                    # [128, NCHUNK*8]

        g = inv[cfg.storage_range(r)]
        dv = np.zeros(cfg.NPAD, dtype=np.float32)
        dv[:LN] = dinv[g]
        dinvc = np.ascontiguousarray(dv.reshape(NBLK, 128).T)  # [128, NBLK]

        # layer-2 E: E[slot, j] = dinv_dst  (src dinv lives in the table)
        blks = np.array([b for (b, h) in schedule])
        dvblk = dinvc.T[blks]                        # [NCHUNK, 128]
        E = (dl_arr[:, :, None] == cols[None, None, :]).astype(np.float16)
        E *= dvblk[:, None, :].astype(np.float16)
        eall = np.ascontiguousarray(
            E.transpose(1, 0, 2).reshape(128, NCHUNK * 128))

        # ---- layer-1 xg + E1 arrays (chunk-slot order, x rows duplicated)
        src1_arr = np.zeros((NCHUNK1, 128), dtype=np.int64)
        dl1_arr = np.full((NCHUNK1, 128), -1.0, dtype=np.float32)
        sc1_arr = np.zeros((NCHUNK1, 128), dtype=np.float32)  # dinv_src
        pos_in1 = {}
        for ci, b in enumerate(schedule1):
            k = pos_in1.get(b, 0)
            pos_in1[b] = k + 1
            kk = r * NBLK + b
            lo, hi = starts1[kk], starts1[kk + 1]
            a = lo + k * 128
            nreal = max(0, min(128, hi - a))
            if nreal > 0:
                seg = slice(a, a + nreal)
                src1_arr[ci, :nreal] = s1_src[seg]
                dl1_arr[ci, :nreal] = s1_dloc[seg]
                sc1_arr[ci, :nreal] = dinv_stor[s1_src[seg]]
        # xg: [slot, chunk, ch] -> [128, NCHUNK1*128] (pure row duplication)
        xg = x_stor[src1_arr.reshape(-1)].astype(np.float16)
        xg = np.ascontiguousarray(
            xg.reshape(NCHUNK1, 128, 128).transpose(1, 0, 2)
            .reshape(128, NCHUNK1 * 128))
        # E1: full norm dinv_src * dinv_dst
        blks1 = np.array(schedule1)
        dvblk1 = dinvc.T[blks1]                      # [NCHUNK1, 128]
        E1 = (dl1_arr[:, :, None] == cols[None, None, :]).astype(np.float32)
        E1 *= sc1_arr[:, :, None] * dvblk1[:, None, :]
        e1all = np.ascontiguousarray(
            E1.astype(np.float16).transpose(1, 0, 2)
            .reshape(128, NCHUNK1 * 128))

        # self-loop diag(dinv) per block
        sd = np.zeros((128, NBLK * 128), dtype=np.float16)
        for b in range(NBLK):
            np.fill_diagonal(sd[:, b * 128:(b + 1) * 128],
                             dinvc[:, b].astype(np.float16))

        # layer-3 weighted-reduce coefficients, broadcast across partitions
        qv = np.zeros(cfg.NPAD, dtype=np.float32)
        qv[:LN] = q_stor[cfg.storage_range(r)]
        qdall = np.ascontiguousarray(
            np.broadcast_to(qv[None, :], (128, cfg.NPAD)))

        xs = x_stor[cfg.storage_range(r)]
        xT = np.zeros((128, cfg.NPAD), dtype=np.float16)
        xT[:, :LN] = xs.T.astype(np.float16)

        per_core.append(dict(xT=xT, idxw=idxw, dinvc=dinvc, eall=eall,
                             xg=xg, e1all=e1all, sd=sd, qdall=qdall))

    meta = dict(CL=CL.astype(int).tolist(), CH=CH.astype(int).tolist(),
                C1=C1.astype(int).tolist(), groups=groups, groups1=groups1,
                schedule=schedule, schedule1=schedule1,
                NCHUNK=NCHUNK, NCHUNK1=NCHUNK1)
    return meta, per_core, dinv


SINGLE_PACKET = False
USE_PREP = False


def build_program(cfg, meta):
    import concourse.bass as bass
    import concourse.bacc as bacc
    import concourse.tile as tile
    from concourse import mybir
    from contextlib import ExitStack

    f32 = mybir.dt.float32
    f16 = mybir.dt.float16
    i16 = mybir.dt.int16
    N, C, M, LN = cfg.N, cfg.C, cfg.M, cfg.LN
    NBLK, NPAD, SPLIT = cfg.NBLK, cfg.NPAD, cfg.SPLIT

    def blk_width(b):
        if b < cfg.NBLK_LO:
            return min(128, cfg.LO_PER - b * 128)
        return min(128, cfg.HI_PER - (b - cfg.NBLK_LO) * 128)

    def blk_slab_row(b):
        if b < cfg.NBLK_LO:
            return 0, b * 128
        return 1, (b - cfg.NBLK_LO) * 128

    CL, CH, C1 = meta["CL"], meta["CH"], meta["C1"]
    groups, groups1 = meta["groups"], meta["groups1"]
    schedule, schedule1 = meta["schedule"], meta["schedule1"]
    NCHUNK, NCHUNK1 = meta["NCHUNK"], meta["NCHUNK1"]

    total_chunks = [CL[b] + CH[b] for b in range(NBLK)]
    last_grp1 = len(groups1) - 1

    nc = bacc.Bacc(None, target_bir_lowering=False, debug=False)
    xT_e = nc.declare_dram_parameter("xT", [128, NPAD], f16, isOutput=False)
    idx_e = nc.declare_dram_parameter("idxw", [128, NCHUNK * 8], i16,
                                      isOutput=False)
    dinvc_e = nc.declare_dram_parameter("dinvc", [128, NBLK], f32,
                                        isOutput=False)
    eall_e = nc.declare_dram_parameter("eall", [128, NCHUNK * 128], f16,
                                       isOutput=False)
    xg_e = nc.declare_dram_parameter("xg", [128, NCHUNK1 * 128], f16,
                                     isOutput=False)
    e1all_e = nc.declare_dram_parameter("e1all", [128, NCHUNK1 * 128], f16,
                                        isOutput=False)
    sd_e = nc.declare_dram_parameter("sd", [128, NBLK * 128], f16,
                                     isOutput=False)
    qdall_e = nc.declare_dram_parameter("qdall", [128, NPAD], f32,
                                        isOutput=False)
    W_e = [nc.declare_dram_parameter(f"W{i}", [C, C], f32, isOutput=False)
           for i in (1, 2, 3)]
    b_e = [nc.declare_dram_parameter(f"b{i}", [C, 1], f32, isOutput=False)
           for i in (1, 2)]
    out_e = nc.declare_dram_parameter("out_vec", [1, C], f32, isOutput=True)

    with TileCtx(nc, tile) as tc, ExitStack() as ctx:
        const = ctx.enter_context(tc.tile_pool(name="const", bufs=1))
        dram = ctx.enter_context(tc.tile_pool(name="dram", bufs=1,
                                              space="DRAM"))
        gpool = ctx.enter_context(tc.tile_pool(name="gath", bufs=2))
        epool = ctx.enter_context(tc.tile_pool(name="esb", bufs=2))
        spool = ctx.enter_context(tc.tile_pool(name="strm", bufs=3))
        hpool = ctx.enter_context(tc.tile_pool(name="hsb", bufs=3))
        tpool = ctx.enter_context(tc.tile_pool(name="tsb", bufs=2))
        mpool = ctx.enter_context(tc.tile_pool(name="tmp", bufs=3))
        psA = ctx.enter_context(tc.tile_pool(name="psA", bufs=4,
                                             space="PSUM"))
        psP = ctx.enter_context(tc.tile_pool(name="psP", bufs=2,
                                             space="PSUM"))
        psZ = ctx.enter_context(tc.tile_pool(name="psZ", bufs=2,
                                             space="PSUM"))

        xT_sb = const.tile([128, NPAD], f16)
        nc.sync.dma_start(out=xT_sb[:], in_=xT_e[:])
        idx_sb = const.tile([128, NCHUNK * 8], i16)
        nc.sync.dma_start(out=idx_sb[:], in_=idx_e[:])
        dinvc_sb = const.tile([128, NBLK], f32)
        nc.sync.dma_start(out=dinvc_sb[:], in_=dinvc_e[:])
        sd_sb = const.tile([128, NBLK * 128], f16)
        nc.sync.dma_start(out=sd_sb[:], in_=sd_e[:])
        qdall_sb = const.tile([128, NPAD], f32)
        nc.sync.dma_start(out=qdall_sb[:], in_=qdall_e[:])
        W_sb = []
        for i in range(3):
            w = const.tile([128, 128], f32, name=f"w{i}_sb")
            nc.sync.dma_start(out=w[:], in_=W_e[i][:])
            W_sb.append(w)
        # f16 copy of W1 (lhsT of the per-block W1-apply matmul)
        w1h_sb = const.tile([128, 128], f16, name="w1h_sb")
        nc.scalar.activation(out=w1h_sb[:], in_=W_sb[0][:],
                             func=mybir.ActivationFunctionType.Copy)
        bias_sb = []
        for i in range(2):
            bcol = const.tile([128, 1], f32, name=f"b{i}_sb")
            nc.sync.dma_start(out=bcol[:], in_=b_e[i][:])
            bias_sb.append(bcol)
        parts_sb = const.tile([128, NBLK], f32)
        nc.vector.memset(parts_sb[:], 0.0)

        # NOTE: gather tables are Local (not Shared) and exactly sized with
        # zero AP offset — dma_gather's Q7 descriptor generator crashes the
        # device on Shared-scratchpad or offset-view sources.
        slab_lo = dram.tile([cfg.LO_PER, C], f16)
        t_lo_full = dram.tile([SPLIT, C], f16, name="t_lo_l2")
        if cfg.HI_PER:
            slab_hi = dram.tile([cfg.HI_PER, C], f16)
            t_hi_full = dram.tile([N - SPLIT, C], f16, name="t_hi_l2")

        rg = [list(range(M))]
        tsb_keep = {}

        def t_block(b, lhsT_ap, w_sb, write_slab):
            """t_blk = dinv * (h_blk @ W); keep in SBUF, optionally to slab."""
            z_ps = psZ.tile([128, 128], f32, tag="zps")
            nc.tensor.matmul(out=z_ps[:], lhsT=lhsT_ap, rhs=w_sb[:],
                             start=True, stop=True)
            t_sb = tpool.tile([128, 128], f16, tag=f"tk{b}")
            nc.scalar.activation(out=t_sb[:], in_=z_ps[:],
                                 func=mybir.ActivationFunctionType.Copy,
                                 scale=dinvc_sb[:, b:b + 1])
            tsb_keep[b] = t_sb
            if write_slab:
                w = blk_width(b)
                which, row = blk_slab_row(b)
                slab = slab_lo if which == 0 else slab_hi
                nc.sync.dma_start(out=slab[row:row + w, :], in_=t_sb[:w, :])

        def emit_allgather(which, piece=None):
            if which == 0:
                # piece-major lo table: piece k holds [8 cores x pc rows]
                pc = cfg.LO_PER // cfg.LO_PIECES
                k = piece
                nc.gpsimd.collective_compute(
                    "AllGather", mybir.AluOpType.bypass, replica_groups=rg,
                    ins=[slab_lo[k * pc:(k + 1) * pc, :]],
                    outs=[t_lo_full[k * pc * M:(k + 1) * pc * M, :]])
            elif cfg.HI_PER:
                nc.gpsimd.collective_compute(
                    "AllGather", mybir.AluOpType.bypass, replica_groups=rg,
                    ins=[slab_hi[:]], outs=[t_hi_full[:]])

        # ---------------- layer 1: x-space scatter, no gather -----------
        # t1 blocks for self-loops (no slab write; no AllGather of t1)
        for b in range(NBLK):
            t_block(b, xT_sb[:, b * 128:(b + 1) * 128], w1h_sb,
                    write_slab=False)

        ci1 = 0
        for gi, grp in enumerate(groups1):
            nck = sum(C1[b] for b in grp)
            pchain = {}
            if nck > 0:
                c0 = ci1
                xg_sb = spool.tile([128, nck * 128], f16, tag="xg")
                nc.sync.dma_start(
                    out=xg_sb[:], in_=xg_e[:, c0 * 128:(c0 + nck) * 128])
                e1_sb = epool.tile([128, nck * 128], f16, tag="e1")
                nc.sync.dma_start(
                    out=e1_sb[:], in_=e1all_e[:, c0 * 128:(c0 + nck) * 128])
                # per-block P chains in x-space
                for j in range(nck):
                    b = schedule1[c0 + j]
                    if b not in pchain:
                        pchain[b] = psP.tile([128, 128], f32, tag="pps",
                                             name=f"p_b{b}")
                    nc.tensor.matmul(
                        out=pchain[b][:],
                        lhsT=xg_sb[:, j * 128:(j + 1) * 128],
                        rhs=e1_sb[:, j * 128:(j + 1) * 128],
                        start=(j == 0 or schedule1[c0 + j - 1] != b),
                        stop=(j == nck - 1 or schedule1[c0 + j + 1] != b))
                ci1 += nck
            for b in grp:
                # aggT_b = (diag(dinv) t1_b).T + W1^T P_b
                w = blk_width(b)
                agg = psA.tile([128, 128], f32, tag="agg", name=f"agg1_{b}")
                has_p = C1[b] > 0
                nc.tensor.matmul(
                    out=agg[:], lhsT=tsb_keep[b][:w, :],
                    rhs=sd_sb[:w, b * 128:b * 128 + 128],
                    start=True, stop=not has_p)
                if has_p:
                    pc_sb = mpool.tile([128, 128], f16, tag="pc")
                    nc.scalar.activation(
                        out=pc_sb[:], in_=pchain[b][:],
                        func=mybir.ActivationFunctionType.Copy)
                    nc.tensor.matmul(
                        out=agg[:], lhsT=w1h_sb[:], rhs=pc_sb[:],
                        start=False, stop=True)
                # epilogue: h1 = relu(agg + b1); t2 slab
                h_sb = hpool.tile([128, 128], f32, tag="hsb")
                nc.scalar.activation(
                    out=h_sb[:], in_=agg[:],
                    func=mybir.ActivationFunctionType.Relu,
                    bias=bias_sb[0][:])
                t_block(b, h_sb[:], W_sb[1], write_slab=True)
            bpp = cfg.LO_PER // cfg.LO_PIECES // 128  # blocks per lo piece
            for k in range(cfg.LO_PIECES):
                if grp[-1] == (k + 1) * bpp - 1:
                    emit_allgather(0, piece=k)
            if gi == last_grp1:
                emit_allgather(1)

        # ---------------- layer 2: gather + scatter -----------------
        chain_pos = [0] * NBLK
        ci = 0
        for gi, grp in enumerate(groups):
            agg_tiles = {}
            for b in grp:
                w = blk_width(b)
                agg_tiles[b] = psA.tile([128, 128], f32, tag="agg",
                                        name=f"agg2_{b}")
                nc.tensor.matmul(
                    out=agg_tiles[b][:], lhsT=tsb_keep[b][:w, :],
                    rhs=sd_sb[:w, b * 128:b * 128 + 128],
                    start=True, stop=(total_chunks[b] == 0))
            n_lo = sum(CL[b] for b in grp)
            n_hi = sum(CH[b] for b in grp)
            for half, nck in ((0, n_lo), (1, n_hi)):
                if nck == 0:
                    continue
                c0 = ci
                c0 = ci
                src_view = t_lo_full[:] if half == 0 else t_hi_full[:]
                e_sb = epool.tile([128, nck * 128], f16, tag=f"e{half}")
                nc.sync.dma_start(
                    out=e_sb[:],
                    in_=eall_e[:, c0 * 128:(c0 + nck) * 128])
                if USE_PREP:
                    # prepare_only + per-piece trigger: the engine pays only
                    # descriptor generation, not DMA-completion wait.  The
                    # rotating per-piece tiles bound prep runahead (WAR) so
                    # the SWDGE descriptor ring (~16K descs) cannot overflow.
                    PIECE = 12
                    for p0 in range(0, nck, PIECE):
                        p1 = min(p0 + PIECE, nck)
                        gtp = gpool.tile([128, PIECE, 128], f16,
                                         tag=f"p{(prep_rot[0]) % 3}")
                        prep_rot[0] += 1
                        nc.gpsimd.dma_gather(
                            gtp[:, :p1 - p0, :], src_view,
                            idx_sb[:, (c0 + p0) * 8:(c0 + p1) * 8],
                            num_idxs=(p1 - p0) * 128,
                            num_idxs_reg=(p1 - p0) * 128,
                            elem_size=C, single_packet=False,
                            prepare_only=True, sem=gsem)
                        nc.gpsimd.trigger_dma(count=None)
                        for j in range(p0, p1):
                            b, h = schedule[c0 + j]
                            nc.tensor.matmul(
                                out=agg_tiles[b][:],
                                lhsT=gtp[:, j - p0, :],
                                rhs=e_sb[:, j * 128:(j + 1) * 128],
                                start=False,
                                stop=(chain_pos[b] == total_chunks[b] - 1))
                            chain_pos[b] += 1
                else:
                    gt = gpool.tile([128, nck, 128], f16, tag=f"g{half}")
                    nc.gpsimd.dma_gather(
                        gt[:], src_view,
                        idx_sb[:, c0 * 8:(c0 + nck) * 8],
                        num_idxs=nck * 128, num_idxs_reg=nck * 128,
                        elem_size=C, single_packet=False)
                    for j in range(nck):
                        b, h = schedule[c0 + j]
                        nc.tensor.matmul(
                            out=agg_tiles[b][:],
                            lhsT=gt[:, j, :],
                            rhs=e_sb[:, j * 128:(j + 1) * 128],
                            start=False,
                            stop=(chain_pos[b] == total_chunks[b] - 1))
                        chain_pos[b] += 1
                ci += nck
            # epilogue: h2 = relu(agg + b2); weighted reduce with q
            for b in grp:
                h_sb = hpool.tile([128, 128], f32, tag="hsb")
                nc.scalar.activation(
                    out=h_sb[:], in_=agg_tiles[b][:],
                    func=mybir.ActivationFunctionType.Relu,
                    bias=bias_sb[1][:])
                qh = mpool.tile([128, 128], f32, tag="qh")
                nc.vector.tensor_tensor(
                    out=qh[:], in0=h_sb[:],
                    in1=qdall_sb[:, b * 128:(b + 1) * 128],
                    op=mybir.AluOpType.mult)
                nc.vector.reduce_sum(
                    out=parts_sb[:, b:b + 1], in_=qh[:],
                    axis=mybir.AxisListType.X)

        # ---------------- layer 3 (collapsed): v @ W3 ----------------
        v_sb = const.tile([128, 1], f32, name="v_sb")
        nc.vector.reduce_sum(out=v_sb[:], in_=parts_sb[:],
                             axis=mybir.AxisListType.X)
        o_ps = psZ.tile([1, 128], f32, tag="zps")
        nc.tensor.matmul(out=o_ps[:], lhsT=v_sb[:], rhs=W_sb[2][:],
                         start=True, stop=True)
        o_sb = const.tile([1, 128], f32, name="o_sb")
        nc.scalar.activation(out=o_sb[:], in_=o_ps[:],
                             func=mybir.ActivationFunctionType.Copy)
        nc.sync.dma_start(out=out_e[:], in_=o_sb[:])

    nc.compile()
    return nc


# small helper so build_program can use `with TileCtx(...)`
def TileCtx(nc, tile_mod):
    return tile_mod.TileContext(nc)


def run(cfg, meta, per_core, weights, trace=False):
    from concourse.bass_utils import run_bass_kernel_spmd

    nc = build_program(cfg, meta)
    in_maps = []
    for r in range(cfg.M):
        m = dict(per_core[r])
        m["W1"], m["W2"], m["W3"] = weights["W1"], weights["W2"], weights["W3"]
        m["b1"] = weights["b1"].reshape(cfg.C, 1).astype(np.float32)
        m["b2"] = weights["b2"].reshape(cfg.C, 1).astype(np.float32)
        in_maps.append(m)
    res = run_bass_kernel_spmd(nc, in_maps, core_ids=list(range(cfg.M)),
                               trace=trace)
    return res


def kernel(**inputs):
    cfg = GCNConfig()
    x = np.asarray(inputs["x"], dtype=np.float32)
    meta, per_core, dinv = host_prep(cfg, x, inputs["edge_index"])
    weights = {k: np.asarray(inputs[k], dtype=np.float32)
               for k in ("W1", "b1", "W2", "b2", "W3", "b3")}
    res = run(cfg, meta, per_core, weights, trace=False)
    total = np.zeros(cfg.C, dtype=np.float64)
    for r in range(cfg.M):
        total += res.results[r]["out_vec"].astype(np.float64).reshape(-1)
    out = total / cfg.N + weights["b3"].astype(np.float64)
    return out.astype(np.float32)
